# revision 19
# baseline (speedup 1.0000x reference)
"""Trainium2 Bass kernel for nn_EnsembleDynamicModel.

Ensemble MLP: E=7 members, x=[state(32)|action(8)] -> 256 -> 256 -> 256 -> 128
-> {mu(32), log_sigma(32)} with swish hidden activations, soft-clamped
log_sigma -> sigma=exp(.), and mu += state residual.

Strategy: data-parallel over the batch axis. Each of the 8 NeuronCores gets
B/8 = 4096 batch rows; ensemble weights are replicated. On-chip layout keeps
activations feature-major ([feature, batch]) so the contraction dim of every
GEMM sits on SBUF partitions:

    h_out[o, n] = sum_k W[k, o] * h_in[k, n]   (lhsT = W tile, rhs = h_in)

The host pre-transposes x once (cheap, 5 MB) and post-transposes the two
[E*32, B/8] outputs back.

Engines:
  PE   : whole GEMM chain. Storage dtype for weights/activations is bf16 by
         default (full-rate 1 column/cycle at the warm 2.4 GHz clock + fast
         weight load); fp32 storage with float32r matmul views is the
         higher-precision fallback (~427ns per 512-col matmul, SBUF-BW
         limited). PSUM accumulation is fp32 either way.
  ACT  : swish fused with the bias add (Silu(psum + b)); the sigma head's
         sigmoid runs as Tanh (same ACT table set as Silu, so the function
         table never swaps): sigmoid(z) = 0.5*tanh(z/2) + 0.5.
  DVE  : one fused affine_then_add drains each head psum (mu = psum + bmu +
         state on rows 0-31, sigma-preact + (bsig-max) on rows 32-63), plus
         the final sigma scale/offset.

The reference's soft_clamp+exp collapses exactly:
    sigma = exp(min) + exp(max) * sigmoid(y - max).

Ensembles are software-pipelined: L0(e+1) is emitted between L3(e) and
head(e) so the PE always has independent matmul work while head psums drain.
"""

import os
import sys
import numpy as np
from contextlib import ExitStack

# concourse ships with the container image (also on PYTHONPATH via axon_site).
for _p in ("/opt/trn_rl_repo", "/root/.axon_site/_ro/trn_rl_repo"):
    if os.path.isdir(_p) and _p not in sys.path:
        sys.path.append(_p)

import ml_dtypes  # noqa: E402
import concourse.bass as bass  # noqa: E402
import concourse.tile as tile  # noqa: E402
import concourse.mybir as mybir  # noqa: E402
from concourse import bacc  # noqa: E402
from concourse.bass_utils import run_bass_kernel_spmd  # noqa: E402
from concourse import bass_utils as _bu  # noqa: E402

USE_BF16 = True

# Consecutive matmuls here reuse the same stationary weights 4x; walrus's
# LDWEIGHTS dedup (off by default) removes the redundant reloads. Only safe
# for 4-byte weight loads — bf16's fast-weight-load path rejects the opt.
if not USE_BF16 and not getattr(_bu, "_ldw_opt_patched", False):
    _orig_run_command = _bu.run_command

    def _run_command_ldw(argv, **kw):
        argv = ["--enable-ldw-opt=true" if a == "--enable-ldw-opt=false" else a
                for a in argv]
        return _orig_run_command(argv, **kw)

    _bu.run_command = _run_command_ldw
    _bu._ldw_opt_patched = True

F32 = mybir.dt.float32
AF = mybir.ActivationFunctionType

if USE_BF16:
    STORE = mybir.dt.bfloat16      # weights + hidden activations storage
    NP_STORE = ml_dtypes.bfloat16
    _mmv = lambda ap: ap           # matmul reads the tiles natively
else:
    STORE = F32                    # fp32 storage, float32r matmul views
    NP_STORE = np.float32
    _mmv = lambda ap: ap.bitcast(mybir.dt.float32r)

E = 7
B = 32768
S = 32
A = 8
DIN = S + A            # 40
NCORES = 8
BL = B // NCORES       # 4096 batch rows per core
CH = 2048              # batch chunk per psum tile (4 PSUM banks fp32)
NSUB = 512             # one matmul's free dim (1 PSUM bank fp32)
NCHUNK = BL // CH      # 2
NJ = CH // NSUB        # 4
NCONST = 8             # const columns per ensemble member


def _build_kernel(ctx, tc, io, act=AF.Silu):
    nc = tc.nc
    cpool = ctx.enter_context(tc.tile_pool(name="cpool", bufs=1))
    hpool = ctx.enter_context(tc.tile_pool(name="hpool", bufs=1))
    wpool = ctx.enter_context(tc.tile_pool(name="wpool", bufs=2))
    pspool = ctx.enter_context(tc.tile_pool(name="pspool", bufs=2, space="PSUM"))
    sgpool = ctx.enter_context(tc.tile_pool(name="sgpool", bufs=3))

    def load_weights(e, first=False):
        w0 = wpool.tile([DIN, 256], STORE, tag="w0", name="w0")
        nc.sync.dma_start(_mmv(w0[:]), _mmv(io["w0"][e]))
        if first:
            # startup: xt chunk 0 right after w0 so L0 can begin ASAP
            for j in range(NJ):
                js = slice(j * NSUB, (j + 1) * NSUB)
                nc.sync.dma_start(_mmv(xt[:, js]), _mmv(io["xt"][:, js]))
            nc.sync.dma_start(cns[:], io["cns"])
            nc.sync.dma_start(sgc[:], io["sgc"])
        w1, w2, w3 = [], [], []
        for k in range(2):
            t = wpool.tile([128, 256], STORE, tag=f"w1_{k}", name=f"w1_{k}")
            nc.sync.dma_start(_mmv(t[:]),
                              _mmv(io["w1"][e, k * 128:(k + 1) * 128, :]))
            w1.append(t)
            t = wpool.tile([128, 256], STORE, tag=f"w2_{k}", name=f"w2_{k}")
            nc.sync.dma_start(_mmv(t[:]),
                              _mmv(io["w2"][e, k * 128:(k + 1) * 128, :]))
            w2.append(t)
            t = wpool.tile([128, 128], STORE, tag=f"w3_{k}", name=f"w3_{k}")
            nc.sync.dma_start(_mmv(t[:]),
                              _mmv(io["w3"][e, k * 128:(k + 1) * 128, :]))
            w3.append(t)
        wh = wpool.tile([128, 64], STORE, tag="wh", name="wh")
        nc.sync.dma_start(_mmv(wh[:]), _mmv(io["wh"][e]))
        if first:
            for j in range(NJ):
                js = slice(CH + j * NSUB, CH + (j + 1) * NSUB)
                nc.sync.dma_start(_mmv(xt[:, js]), _mmv(io["xt"][:, js]))
            nc.sync.dma_start(resid[:], io["resid"])
        return w0, w1, w2, w3, wh

    scratch = cpool.tile([1, 8], F32, tag="scratch")
    nc.gpsimd.memset(scratch[:], 0.0)
    nc.scalar.activation(scratch[0:1, 0:8], scratch[0:1, 0:8], act, bias=0.0)

    xt = cpool.tile([DIN, BL], STORE, tag="xt")
    cns = cpool.tile([128, E * NCONST], F32, tag="cns")
    sgc = cpool.tile([128, 2], F32, tag="sgc")
    resid = cpool.tile([64, BL], F32, tag="resid")

    # --- activation buffers, ping-pong between layers ---
    hA = [hpool.tile([128, BL], STORE, tag=f"hA{i}", name=f"hA{i}")
          for i in range(2)]
    hB = [hpool.tile([128, BL], STORE, tag=f"hB{i}", name=f"hB{i}")
          for i in range(2)]

    def gemm_layer(h_in, w_tiles, m_tiles, h_out, bias_cols, e):
        """h_out[mt][:, :] = act(sum_kt w[kt].T @ h_in[kt] + b)."""
        nkt = len(h_in)
        for c in range(NCHUNK):
            for mt in range(m_tiles):
                ps = pspool.tile([128, CH], F32, tag="ps", name="ps")
                for kt in range(nkt):
                    for j in range(NJ):
                        ncol = slice(c * CH + j * NSUB, c * CH + (j + 1) * NSUB)
                        nc.tensor.matmul(
                            ps[:, j * NSUB:(j + 1) * NSUB],
                            _mmv(w_tiles[kt][:, mt * 128:(mt + 1) * 128]),
                            _mmv(h_in[kt][:, ncol]),
                            start=(kt == 0),
                            stop=(kt == nkt - 1),
                            skip_group_check=True,
                        )
                bcol = e * NCONST + bias_cols[mt]
                nc.scalar.activation(
                    _mmv(h_out[mt][:, c * CH:(c + 1) * CH]), ps[:, :],
                    act, bias=cns[:, bcol:bcol + 1],
                )

    # Software pipeline over ensembles: L0(e+1) is emitted between L3(e)
    # and head(e) so the PE always has independent matmul work while the
    # head psums drain on the DVE.
    w_cur = None
    for e in range(E):
        if e == 0:
            w_cur = load_weights(0, first=True)
            gemm_layer([xt], [w_cur[0]], 2, hA, (0, 1), 0)   # L0 of e=0
        w0, w1, w2, w3, wh = w_cur

        # --- the GEMM chain, feature-major ---
        gemm_layer(hA, w1, 2, hB, (2, 3), e)           # 256  -> 256
        gemm_layer(hB, w2, 2, hA, (4, 5), e)           # 256  -> 256
        gemm_layer(hA, w3, 1, hB, (6,), e)             # 256  -> 128 (hB[0])
        h3 = hB[0]

        def head_chunk(c, nsplit):
            cs = slice(c * CH, (c + 1) * CH)
            ps = pspool.tile([64, CH], F32, tag="ps", name="psh")
            for j in range(NJ):
                ncol = slice(c * CH + j * NSUB, c * CH + (j + 1) * NSUB)
                nc.tensor.matmul(
                    ps[:, j * NSUB:(j + 1) * NSUB],
                    _mmv(wh[:, :]), _mmv(h3[:, ncol]),
                    start=True, stop=True,
                )
            # single fused DVE op drains the whole head psum:
            #   rows 0-31:  mu = psum + bmu + state
            #   rows 32-63: y' = psum + (bsig - max) + 0
            bcol = e * NCONST + 7
            hd = sgpool.tile([64, CH], F32, tag="hd", name="hd")
            nc.vector.affine_then_add(
                hd[:, :], ps[:, :], resid[:, cs], 1.0,
                cns[0:64, bcol:bcol + 1],
            )
            # sigmoid via tanh (same ACT table set as Silu -> no table swap):
            # sigmoid(y') = 0.5*tanh(y'/2) + 0.5
            sg = sgpool.tile([64, CH], F32, tag="sg", name="sg")
            nc.scalar.activation(sg[32:64, :], hd[32:64, :], AF.Tanh, scale=0.5)
            # sigma = tanh_out*(exp(max)/2) + (exp(min) + exp(max)/2)
            sig = sgpool.tile([64, CH], F32, tag="sig", name="sig")
            nc.vector.tensor_scalar(
                sig[32:64, :], sg[32:64, :],
                sgc[32:64, 0:1], sgc[32:64, 1:2],
                mybir.AluOpType.mult, mybir.AluOpType.add,
            )
            # split the output DMAs so the kernel tail isn't one long transfer
            step = CH // nsplit
            for p in range(nsplit):
                pcs = slice(c * CH + p * step, c * CH + (p + 1) * step)
                pls = slice(p * step, (p + 1) * step)
                nc.sync.dma_start(io["mu"][e * 32:(e + 1) * 32, pcs],
                                  hd[0:32, pls])
                nc.sync.dma_start(io["sig"][e * 32:(e + 1) * 32, pcs],
                                  sig[32:64, pls])

        # The reference's soft_clamp+exp collapses exactly to
        #   sigma = exp(min) + exp(max)*sigmoid(y - max)
        # head chunk 0 is emitted before L0(e+1), chunk 1 after, so the
        # ACT-side tanh never waits long and the PE never starves.
        nsplit = 4 if e == E - 1 else 1
        head_chunk(0, nsplit)
        if e + 1 < E:
            w_cur = load_weights(e + 1)
            gemm_layer([xt], [w_cur[0]], 2, hA, (0, 1), e + 1)  # L0 of e+1
        head_chunk(1, nsplit)


def build_program(act=AF.Silu):
    nc = bacc.Bacc(
        "TRN2", target_bir_lowering=False, debug=False, num_devices=NCORES
    )
    io = {
        "xt": nc.dram_tensor("xt", [DIN, BL], STORE,
                             kind="ExternalInput").ap(),
        "resid": nc.dram_tensor("resid", [64, BL], F32,
                                kind="ExternalInput").ap(),
        "w0": nc.dram_tensor("w0", [E, DIN, 256], STORE,
                             kind="ExternalInput").ap(),
        "w1": nc.dram_tensor("w1", [E, 256, 256], STORE,
                             kind="ExternalInput").ap(),
        "w2": nc.dram_tensor("w2", [E, 256, 256], STORE,
                             kind="ExternalInput").ap(),
        "w3": nc.dram_tensor("w3", [E, 256, 128], STORE,
                             kind="ExternalInput").ap(),
        "wh": nc.dram_tensor("wh", [E, 128, 64], STORE,
                             kind="ExternalInput").ap(),
        "cns": nc.dram_tensor("cns", [128, E * NCONST], F32,
                              kind="ExternalInput").ap(),
        "sgc": nc.dram_tensor("sgc", [128, 2], F32, kind="ExternalInput").ap(),
        "mu": nc.dram_tensor("mu", [E * 32, BL], F32,
                             kind="ExternalOutput").ap(),
        "sig": nc.dram_tensor("sig", [E * 32, BL], F32,
                              kind="ExternalOutput").ap(),
    }
    with tile.TileContext(nc) as tc, ExitStack() as ctx:
        _build_kernel(ctx, tc, io, act=act)
    nc.compile()
    return nc


def host_prep(state, action, W0, b0, W1, b1, W2, b2, W3, b3,
              Wmu, bmu, Wsig, bsig, max_logstd, min_logstd):
    """Full inputs -> (shared input map, per-core shard maps)."""
    f = lambda a: np.ascontiguousarray(np.asarray(a), dtype=np.float32)
    g = lambda a: np.ascontiguousarray(np.asarray(a, dtype=np.float32)
                                       .astype(NP_STORE))
    state, action = f(state), f(action)
    xt_full = np.ascontiguousarray(
        np.concatenate([state, action], axis=1).T
    )  # [40, B] fp32
    wh = np.concatenate([f(Wmu), f(Wsig)], axis=2)
    b0, b1, b2, b3 = f(b0), f(b1), f(b2), f(b3)
    bmu, bsig = f(bmu), f(bsig)
    mx, mn = f(max_logstd), f(min_logstd)

    cns = np.zeros((128, E * NCONST), np.float32)
    for e in range(E):
        c = e * NCONST
        cns[:, c + 0] = b0[e, :128]
        cns[:, c + 1] = b0[e, 128:]
        cns[:, c + 2] = b1[e, :128]
        cns[:, c + 3] = b1[e, 128:]
        cns[:, c + 4] = b2[e, :128]
        cns[:, c + 5] = b2[e, 128:]
        cns[:, c + 6] = b3[e, :]
        cns[0:32, c + 7] = bmu[e]
        cns[32:64, c + 7] = bsig[e] - mx   # sigma-head drain bias

    sgc = np.zeros((128, 2), np.float32)
    sgc[32:64, 0] = np.exp(mx) / 2
    sgc[32:64, 1] = np.exp(mn) + np.exp(mx) / 2

    shared = {
        "w0": g(W0), "w1": g(W1), "w2": g(W2), "w3": g(W3), "wh": g(wh),
        "cns": cns, "sgc": sgc,
    }
    resid_full = np.zeros((64, B), np.float32)
    resid_full[0:32] = xt_full[0:32]
    xt_store = xt_full.astype(NP_STORE)
    shards = [
        {
            "xt": np.ascontiguousarray(xt_store[:, c * BL:(c + 1) * BL]),
            "resid": np.ascontiguousarray(resid_full[:, c * BL:(c + 1) * BL]),
        }
        for c in range(NCORES)
    ]
    return shared, shards


def host_post(results):
    """Per-core {mu,sig} [E*32, BL] -> (mu [E,B,32], sigma [E,B,32])."""
    mu = np.empty((E, B, 32), np.float32)
    sigma = np.empty((E, B, 32), np.float32)
    for c in range(NCORES):
        bs = slice(c * BL, (c + 1) * BL)
        mu[:, bs, :] = results[c]["mu"].reshape(E, 32, BL).transpose(0, 2, 1)
        sigma[:, bs, :] = results[c]["sig"].reshape(E, 32, BL).transpose(0, 2, 1)
    return mu, sigma


_PROGRAM = None


def _get_program():
    global _PROGRAM
    if _PROGRAM is None:
        _PROGRAM = build_program()
    return _PROGRAM


def kernel(**inputs):
    nc = _get_program()
    shared, shards = host_prep(**inputs)
    in_maps = [{**shared, **shards[c]} for c in range(NCORES)]
    res = run_bass_kernel_spmd(nc, in_maps, list(range(NCORES)))
    return host_post(res.results)


# revision 20
# speedup vs baseline: 1.0124x; 1.0124x over previous
"""Trainium2 Bass kernel for nn_EnsembleDynamicModel.

Ensemble MLP: E=7 members, x=[state(32)|action(8)] -> 256 -> 256 -> 256 -> 128
-> {mu(32), log_sigma(32)} with swish hidden activations, soft-clamped
log_sigma -> sigma=exp(.), and mu += state residual.

Strategy: data-parallel over the batch axis. Each of the 8 NeuronCores gets
B/8 = 4096 batch rows; ensemble weights are replicated. On-chip layout keeps
activations feature-major ([feature, batch]) so the contraction dim of every
GEMM sits on SBUF partitions:

    h_out[o, n] = sum_k W[k, o] * h_in[k, n]   (lhsT = W tile, rhs = h_in)

The host pre-transposes x once (cheap, 5 MB) and post-transposes the two
[E*32, B/8] outputs back.

Engines:
  PE   : whole GEMM chain. Storage dtype for weights/activations is bf16 by
         default (full-rate 1 column/cycle at the warm 2.4 GHz clock + fast
         weight load); fp32 storage with float32r matmul views is the
         higher-precision fallback (~427ns per 512-col matmul, SBUF-BW
         limited). PSUM accumulation is fp32 either way.
  ACT  : swish fused with the bias add (Silu(psum + b)); the sigma head's
         sigmoid runs as Tanh (same ACT table set as Silu, so the function
         table never swaps): sigmoid(z) = 0.5*tanh(z/2) + 0.5.
  DVE  : one fused affine_then_add drains each head psum (mu = psum + bmu +
         state on rows 0-31, sigma-preact + (bsig-max) on rows 32-63), plus
         the final sigma scale/offset.

The reference's soft_clamp+exp collapses exactly:
    sigma = exp(min) + exp(max) * sigmoid(y - max).

Ensembles are software-pipelined: L0(e+1) is emitted between L3(e) and
head(e) so the PE always has independent matmul work while head psums drain.
"""

import os
import sys
import numpy as np
from contextlib import ExitStack

# concourse ships with the container image (also on PYTHONPATH via axon_site).
for _p in ("/opt/trn_rl_repo", "/root/.axon_site/_ro/trn_rl_repo"):
    if os.path.isdir(_p) and _p not in sys.path:
        sys.path.append(_p)

import ml_dtypes  # noqa: E402
import concourse.bass as bass  # noqa: E402
import concourse.tile as tile  # noqa: E402
import concourse.mybir as mybir  # noqa: E402
from concourse import bacc  # noqa: E402
from concourse.bass_utils import run_bass_kernel_spmd  # noqa: E402
from concourse import bass_utils as _bu  # noqa: E402

USE_BF16 = True

# Consecutive matmuls here reuse the same stationary weights 4x; walrus's
# LDWEIGHTS dedup (off by default) removes the redundant reloads. Only safe
# for 4-byte weight loads — bf16's fast-weight-load path rejects the opt.
if not USE_BF16 and not getattr(_bu, "_ldw_opt_patched", False):
    _orig_run_command = _bu.run_command

    def _run_command_ldw(argv, **kw):
        argv = ["--enable-ldw-opt=true" if a == "--enable-ldw-opt=false" else a
                for a in argv]
        return _orig_run_command(argv, **kw)

    _bu.run_command = _run_command_ldw
    _bu._ldw_opt_patched = True

F32 = mybir.dt.float32
AF = mybir.ActivationFunctionType

if USE_BF16:
    STORE = mybir.dt.bfloat16      # weights + hidden activations storage
    NP_STORE = ml_dtypes.bfloat16
    _mmv = lambda ap: ap           # matmul reads the tiles natively
else:
    STORE = F32                    # fp32 storage, float32r matmul views
    NP_STORE = np.float32
    _mmv = lambda ap: ap.bitcast(mybir.dt.float32r)

E = 7
B = 32768
S = 32
A = 8
DIN = S + A            # 40
NCORES = 8
BL = B // NCORES       # 4096 batch rows per core
CH = 2048              # batch chunk per psum tile (4 PSUM banks fp32)
NSUB = 512             # one matmul's free dim (1 PSUM bank fp32)
NCHUNK = BL // CH      # 2
NJ = CH // NSUB        # 4
NCONST = 8             # const columns per ensemble member


def _build_kernel(ctx, tc, io, act=AF.Silu):
    nc = tc.nc
    cpool = ctx.enter_context(tc.tile_pool(name="cpool", bufs=1))
    hpool = ctx.enter_context(tc.tile_pool(name="hpool", bufs=1))
    wpool = ctx.enter_context(tc.tile_pool(name="wpool", bufs=2))
    pspool = ctx.enter_context(tc.tile_pool(name="pspool", bufs=2, space="PSUM"))
    sgpool = ctx.enter_context(tc.tile_pool(name="sgpool", bufs=3))

    def load_weights(e, first=False):
        w0 = wpool.tile([DIN, 256], STORE, tag="w0", name="w0")
        nc.sync.dma_start(_mmv(w0[:]), _mmv(io["w0"][e]))
        if first:
            # startup: xt chunk 0 right after w0 so L0 can begin ASAP
            for j in range(NJ):
                js = slice(j * NSUB, (j + 1) * NSUB)
                nc.sync.dma_start(_mmv(xt[:, js]), _mmv(io["xt"][:, js]))
            nc.sync.dma_start(cns[:], io["cns"])
            nc.sync.dma_start(sgc[:], io["sgc"])
        w1, w2, w3 = [], [], []
        for k in range(2):
            t = wpool.tile([128, 256], STORE, tag=f"w1_{k}", name=f"w1_{k}")
            nc.sync.dma_start(_mmv(t[:]),
                              _mmv(io["w1"][e, k * 128:(k + 1) * 128, :]))
            w1.append(t)
            t = wpool.tile([128, 256], STORE, tag=f"w2_{k}", name=f"w2_{k}")
            nc.sync.dma_start(_mmv(t[:]),
                              _mmv(io["w2"][e, k * 128:(k + 1) * 128, :]))
            w2.append(t)
            t = wpool.tile([128, 128], STORE, tag=f"w3_{k}", name=f"w3_{k}")
            nc.sync.dma_start(_mmv(t[:]),
                              _mmv(io["w3"][e, k * 128:(k + 1) * 128, :]))
            w3.append(t)
        wh = wpool.tile([128, 64], STORE, tag="wh", name="wh")
        nc.sync.dma_start(_mmv(wh[:]), _mmv(io["wh"][e]))
        if first:
            for j in range(NJ):
                js = slice(CH + j * NSUB, CH + (j + 1) * NSUB)
                nc.sync.dma_start(_mmv(xt[:, js]), _mmv(io["xt"][:, js]))
            nc.sync.dma_start(resid[:], io["resid"])
        return w0, w1, w2, w3, wh

    scratch = cpool.tile([1, 8], F32, tag="scratch")
    nc.gpsimd.memset(scratch[:], 0.0)
    nc.scalar.activation(scratch[0:1, 0:8], scratch[0:1, 0:8], act, bias=0.0)

    xt = cpool.tile([DIN, BL], STORE, tag="xt")
    cns = cpool.tile([128, E * NCONST], F32, tag="cns")
    sgc = cpool.tile([128, 2], F32, tag="sgc")
    resid = cpool.tile([64, BL], F32, tag="resid")

    # --- activation buffers, ping-pong between layers ---
    hA = [hpool.tile([128, BL], STORE, tag=f"hA{i}", name=f"hA{i}")
          for i in range(2)]
    hB = [hpool.tile([128, BL], STORE, tag=f"hB{i}", name=f"hB{i}")
          for i in range(2)]

    def gemm_layer(h_in, w_tiles, m_tiles, h_out, bias_cols, e):
        """h_out[mt][:, :] = act(sum_kt w[kt].T @ h_in[kt] + b)."""
        nkt = len(h_in)
        for c in range(NCHUNK):
            for mt in range(m_tiles):
                ps = pspool.tile([128, CH], F32, tag="ps", name="ps")
                for kt in range(nkt):
                    for j in range(NJ):
                        ncol = slice(c * CH + j * NSUB, c * CH + (j + 1) * NSUB)
                        nc.tensor.matmul(
                            ps[:, j * NSUB:(j + 1) * NSUB],
                            _mmv(w_tiles[kt][:, mt * 128:(mt + 1) * 128]),
                            _mmv(h_in[kt][:, ncol]),
                            start=(kt == 0),
                            stop=(kt == nkt - 1),
                            skip_group_check=True,
                        )
                bcol = e * NCONST + bias_cols[mt]
                nc.scalar.activation(
                    _mmv(h_out[mt][:, c * CH:(c + 1) * CH]), ps[:, :],
                    act, bias=cns[:, bcol:bcol + 1],
                )

    # Software pipeline over ensembles: L0(e+1) is emitted between L3(e)
    # and head(e) so the PE always has independent matmul work while the
    # head psums drain on the DVE.
    w_cur = None
    for e in range(E):
        if e == 0:
            w_cur = load_weights(0, first=True)
            gemm_layer([xt], [w_cur[0]], 2, hA, (0, 1), 0)   # L0 of e=0
        w0, w1, w2, w3, wh = w_cur

        # --- the GEMM chain, feature-major ---
        gemm_layer(hA, w1, 2, hB, (2, 3), e)           # 256  -> 256
        gemm_layer(hB, w2, 2, hA, (4, 5), e)           # 256  -> 256
        gemm_layer(hA, w3, 1, hB, (6,), e)             # 256  -> 128 (hB[0])
        h3 = hB[0]

        def head_chunk(c, nsplit):
            cs = slice(c * CH, (c + 1) * CH)
            ps = pspool.tile([64, CH], F32, tag="ps", name="psh")
            for j in range(NJ):
                ncol = slice(c * CH + j * NSUB, c * CH + (j + 1) * NSUB)
                nc.tensor.matmul(
                    ps[:, j * NSUB:(j + 1) * NSUB],
                    _mmv(wh[:, :]), _mmv(h3[:, ncol]),
                    start=True, stop=True,
                )
            # single fused DVE op drains the whole head psum:
            #   rows 0-31:  mu = psum + bmu + state
            #   rows 32-63: y' = psum + (bsig - max) + 0
            bcol = e * NCONST + 7
            hd = sgpool.tile([64, CH], F32, tag="hd", name="hd")
            nc.vector.affine_then_add(
                hd[:, :], ps[:, :], resid[:, cs], 1.0,
                cns[0:64, bcol:bcol + 1],
            )
            # sigmoid via tanh (same ACT table set as Silu -> no table swap):
            # sigmoid(y') = 0.5*tanh(y'/2) + 0.5
            sg = sgpool.tile([64, CH], F32, tag="sg", name="sg")
            nc.scalar.activation(sg[32:64, :], hd[32:64, :], AF.Tanh, scale=0.5)
            # sigma = tanh_out*(exp(max)/2) + (exp(min) + exp(max)/2)
            sig = sgpool.tile([64, CH], F32, tag="sig", name="sig")
            nc.vector.tensor_scalar(
                sig[32:64, :], sg[32:64, :],
                sgc[32:64, 0:1], sgc[32:64, 1:2],
                mybir.AluOpType.mult, mybir.AluOpType.add,
            )
            # split the output DMAs so the kernel tail isn't one long transfer
            step = CH // nsplit
            for p in range(nsplit):
                pcs = slice(c * CH + p * step, c * CH + (p + 1) * step)
                pls = slice(p * step, (p + 1) * step)
                nc.sync.dma_start(io["mu"][e * 32:(e + 1) * 32, pcs],
                                  hd[0:32, pls])
                nc.sync.dma_start(io["sig"][e * 32:(e + 1) * 32, pcs],
                                  sig[32:64, pls])

        # The reference's soft_clamp+exp collapses exactly to
        #   sigma = exp(min) + exp(max)*sigmoid(y - max)
        # head chunk 0 is emitted before L0(e+1), chunk 1 after, so the
        # ACT-side tanh never waits long and the PE never starves.
        nsplit = 2 if e == E - 1 else 1
        head_chunk(0, nsplit)
        if e + 1 < E:
            w_cur = load_weights(e + 1)
            gemm_layer([xt], [w_cur[0]], 2, hA, (0, 1), e + 1)  # L0 of e+1
        head_chunk(1, nsplit)


def build_program(act=AF.Silu):
    nc = bacc.Bacc(
        "TRN2", target_bir_lowering=False, debug=False, num_devices=NCORES
    )
    io = {
        "xt": nc.dram_tensor("xt", [DIN, BL], STORE,
                             kind="ExternalInput").ap(),
        "resid": nc.dram_tensor("resid", [64, BL], F32,
                                kind="ExternalInput").ap(),
        "w0": nc.dram_tensor("w0", [E, DIN, 256], STORE,
                             kind="ExternalInput").ap(),
        "w1": nc.dram_tensor("w1", [E, 256, 256], STORE,
                             kind="ExternalInput").ap(),
        "w2": nc.dram_tensor("w2", [E, 256, 256], STORE,
                             kind="ExternalInput").ap(),
        "w3": nc.dram_tensor("w3", [E, 256, 128], STORE,
                             kind="ExternalInput").ap(),
        "wh": nc.dram_tensor("wh", [E, 128, 64], STORE,
                             kind="ExternalInput").ap(),
        "cns": nc.dram_tensor("cns", [128, E * NCONST], F32,
                              kind="ExternalInput").ap(),
        "sgc": nc.dram_tensor("sgc", [128, 2], F32, kind="ExternalInput").ap(),
        "mu": nc.dram_tensor("mu", [E * 32, BL], F32,
                             kind="ExternalOutput").ap(),
        "sig": nc.dram_tensor("sig", [E * 32, BL], F32,
                              kind="ExternalOutput").ap(),
    }
    with tile.TileContext(nc) as tc, ExitStack() as ctx:
        _build_kernel(ctx, tc, io, act=act)
    nc.compile()
    return nc


def host_prep(state, action, W0, b0, W1, b1, W2, b2, W3, b3,
              Wmu, bmu, Wsig, bsig, max_logstd, min_logstd):
    """Full inputs -> (shared input map, per-core shard maps)."""
    f = lambda a: np.ascontiguousarray(np.asarray(a), dtype=np.float32)
    g = lambda a: np.ascontiguousarray(np.asarray(a, dtype=np.float32)
                                       .astype(NP_STORE))
    state, action = f(state), f(action)
    xt_full = np.ascontiguousarray(
        np.concatenate([state, action], axis=1).T
    )  # [40, B] fp32
    wh = np.concatenate([f(Wmu), f(Wsig)], axis=2)
    b0, b1, b2, b3 = f(b0), f(b1), f(b2), f(b3)
    bmu, bsig = f(bmu), f(bsig)
    mx, mn = f(max_logstd), f(min_logstd)

    cns = np.zeros((128, E * NCONST), np.float32)
    for e in range(E):
        c = e * NCONST
        cns[:, c + 0] = b0[e, :128]
        cns[:, c + 1] = b0[e, 128:]
        cns[:, c + 2] = b1[e, :128]
        cns[:, c + 3] = b1[e, 128:]
        cns[:, c + 4] = b2[e, :128]
        cns[:, c + 5] = b2[e, 128:]
        cns[:, c + 6] = b3[e, :]
        cns[0:32, c + 7] = bmu[e]
        cns[32:64, c + 7] = bsig[e] - mx   # sigma-head drain bias

    sgc = np.zeros((128, 2), np.float32)
    sgc[32:64, 0] = np.exp(mx) / 2
    sgc[32:64, 1] = np.exp(mn) + np.exp(mx) / 2

    shared = {
        "w0": g(W0), "w1": g(W1), "w2": g(W2), "w3": g(W3), "wh": g(wh),
        "cns": cns, "sgc": sgc,
    }
    resid_full = np.zeros((64, B), np.float32)
    resid_full[0:32] = xt_full[0:32]
    xt_store = xt_full.astype(NP_STORE)
    shards = [
        {
            "xt": np.ascontiguousarray(xt_store[:, c * BL:(c + 1) * BL]),
            "resid": np.ascontiguousarray(resid_full[:, c * BL:(c + 1) * BL]),
        }
        for c in range(NCORES)
    ]
    return shared, shards


def host_post(results):
    """Per-core {mu,sig} [E*32, BL] -> (mu [E,B,32], sigma [E,B,32])."""
    mu = np.empty((E, B, 32), np.float32)
    sigma = np.empty((E, B, 32), np.float32)
    for c in range(NCORES):
        bs = slice(c * BL, (c + 1) * BL)
        mu[:, bs, :] = results[c]["mu"].reshape(E, 32, BL).transpose(0, 2, 1)
        sigma[:, bs, :] = results[c]["sig"].reshape(E, 32, BL).transpose(0, 2, 1)
    return mu, sigma


_PROGRAM = None


def _get_program():
    global _PROGRAM
    if _PROGRAM is None:
        _PROGRAM = build_program()
    return _PROGRAM


def kernel(**inputs):
    nc = _get_program()
    shared, shards = host_prep(**inputs)
    in_maps = [{**shared, **shards[c]} for c in range(NCORES)]
    res = run_bass_kernel_spmd(nc, in_maps, list(range(NCORES)))
    return host_post(res.results)


# revision 22
# speedup vs baseline: 1.0472x; 1.0343x over previous
"""Trainium2 Bass kernel for nn_EnsembleDynamicModel.

Ensemble MLP: E=7 members, x=[state(32)|action(8)] -> 256 -> 256 -> 256 -> 128
-> {mu(32), log_sigma(32)} with swish hidden activations, soft-clamped
log_sigma -> sigma=exp(.), and mu += state residual.

Strategy: data-parallel over the batch axis. Each of the 8 NeuronCores gets
B/8 = 4096 batch rows; ensemble weights are replicated. On-chip layout keeps
activations feature-major ([feature, batch]) so the contraction dim of every
GEMM sits on SBUF partitions:

    h_out[o, n] = sum_k W[k, o] * h_in[k, n]   (lhsT = W tile, rhs = h_in)

The host pre-transposes x once (cheap, 5 MB) and post-transposes the two
[E*32, B/8] outputs back.

Engines:
  PE   : whole GEMM chain. Storage dtype for weights/activations is bf16 by
         default (full-rate 1 column/cycle at the warm 2.4 GHz clock + fast
         weight load); fp32 storage with float32r matmul views is the
         higher-precision fallback (~427ns per 512-col matmul, SBUF-BW
         limited). PSUM accumulation is fp32 either way.
  ACT  : swish fused with the bias add (Silu(psum + b)); the sigma head's
         sigmoid runs as Tanh (same ACT table set as Silu, so the function
         table never swaps): sigmoid(z) = 0.5*tanh(z/2) + 0.5.
  DVE  : one fused affine_then_add drains each head psum (mu = psum + bmu +
         state on rows 0-31, sigma-preact + (bsig-max) on rows 32-63), plus
         the final sigma scale/offset.

The reference's soft_clamp+exp collapses exactly:
    sigma = exp(min) + exp(max) * sigmoid(y - max).

Ensembles are software-pipelined: L0(e+1) is emitted between L3(e) and
head(e) so the PE always has independent matmul work while head psums drain.
"""

import os
import sys
import numpy as np
from contextlib import ExitStack

# concourse ships with the container image (also on PYTHONPATH via axon_site).
for _p in ("/opt/trn_rl_repo", "/root/.axon_site/_ro/trn_rl_repo"):
    if os.path.isdir(_p) and _p not in sys.path:
        sys.path.append(_p)

import ml_dtypes  # noqa: E402
import concourse.bass as bass  # noqa: E402
import concourse.tile as tile  # noqa: E402
import concourse.mybir as mybir  # noqa: E402
from concourse import bacc  # noqa: E402
from concourse.bass_utils import run_bass_kernel_spmd  # noqa: E402
from concourse import bass_utils as _bu  # noqa: E402

USE_BF16 = True

# Consecutive matmuls here reuse the same stationary weights 4x; walrus's
# LDWEIGHTS dedup (off by default) removes the redundant reloads. Only safe
# for 4-byte weight loads — bf16's fast-weight-load path rejects the opt.
if not USE_BF16 and not getattr(_bu, "_ldw_opt_patched", False):
    _orig_run_command = _bu.run_command

    def _run_command_ldw(argv, **kw):
        argv = ["--enable-ldw-opt=true" if a == "--enable-ldw-opt=false" else a
                for a in argv]
        return _orig_run_command(argv, **kw)

    _bu.run_command = _run_command_ldw
    _bu._ldw_opt_patched = True

F32 = mybir.dt.float32
AF = mybir.ActivationFunctionType

if USE_BF16:
    STORE = mybir.dt.bfloat16      # weights + hidden activations storage
    NP_STORE = ml_dtypes.bfloat16
    _mmv = lambda ap: ap           # matmul reads the tiles natively
else:
    STORE = F32                    # fp32 storage, float32r matmul views
    NP_STORE = np.float32
    _mmv = lambda ap: ap.bitcast(mybir.dt.float32r)

E = 7
B = 32768
S = 32
A = 8
DIN = S + A            # 40
NCORES = 8
BL = B // NCORES       # 4096 batch rows per core
CH = 2048              # batch chunk per psum tile (4 PSUM banks fp32)
NSUB = 512             # one matmul's free dim (1 PSUM bank fp32)
NCHUNK = BL // CH      # 2
NJ = CH // NSUB        # 4
NCONST = 8             # const columns per ensemble member


def _build_kernel(ctx, tc, io, act=AF.Silu):
    nc = tc.nc
    cpool = ctx.enter_context(tc.tile_pool(name="cpool", bufs=1))
    hpool = ctx.enter_context(tc.tile_pool(name="hpool", bufs=1))
    wpool = ctx.enter_context(tc.tile_pool(name="wpool", bufs=2))
    pspool = ctx.enter_context(tc.tile_pool(name="pspool", bufs=2, space="PSUM"))
    sgpool = ctx.enter_context(tc.tile_pool(name="sgpool", bufs=3))

    def load_weights(e, first=False):
        w0 = wpool.tile([DIN, 256], STORE, tag="w0", name="w0")
        nc.sync.dma_start(_mmv(w0[:]), _mmv(io["w0"][e]))
        if first:
            # startup: xt chunk 0 right after w0 so L0 can begin ASAP
            for j in range(NJ):
                js = slice(j * NSUB, (j + 1) * NSUB)
                nc.sync.dma_start(_mmv(xt[:, js]), _mmv(io["xt"][:, js]))
            nc.sync.dma_start(cns[:], io["cns"])
            nc.sync.dma_start(sgc[:], io["sgc"])
        w1, w2, w3 = [], [], []
        for k in range(2):
            t = wpool.tile([128, 256], STORE, tag=f"w1_{k}", name=f"w1_{k}")
            nc.sync.dma_start(_mmv(t[:]),
                              _mmv(io["w1"][e, k * 128:(k + 1) * 128, :]))
            w1.append(t)
            t = wpool.tile([128, 256], STORE, tag=f"w2_{k}", name=f"w2_{k}")
            nc.sync.dma_start(_mmv(t[:]),
                              _mmv(io["w2"][e, k * 128:(k + 1) * 128, :]))
            w2.append(t)
            t = wpool.tile([128, 128], STORE, tag=f"w3_{k}", name=f"w3_{k}")
            nc.sync.dma_start(_mmv(t[:]),
                              _mmv(io["w3"][e, k * 128:(k + 1) * 128, :]))
            w3.append(t)
        wh = wpool.tile([128, 64], STORE, tag="wh", name="wh")
        nc.sync.dma_start(_mmv(wh[:]), _mmv(io["wh"][e]))
        if first:
            for j in range(NJ):
                js = slice(CH + j * NSUB, CH + (j + 1) * NSUB)
                nc.sync.dma_start(_mmv(xt[:, js]), _mmv(io["xt"][:, js]))
            nc.sync.dma_start(resid[:], io["resid"])
        return w0, w1, w2, w3, wh

    scratch = cpool.tile([1, 8], F32, tag="scratch")
    nc.gpsimd.memset(scratch[:], 0.0)
    nc.scalar.activation(scratch[0:1, 0:8], scratch[0:1, 0:8], act, bias=0.0)

    xt = cpool.tile([DIN, BL], STORE, tag="xt")
    cns = cpool.tile([128, E * NCONST], F32, tag="cns")
    sgc = cpool.tile([128, 2], F32, tag="sgc")
    resid = cpool.tile([64, BL], F32, tag="resid")

    # sigma pre-activations packed 4 ensembles per tile: row 32*(e%4)+i
    pk = [sgpool.tile([128, BL], F32, tag=f"pk{g}", name=f"pk{g}", bufs=1)
          for g in range(2)]

    # --- activation buffers, ping-pong between layers ---
    hA = [hpool.tile([128, BL], STORE, tag=f"hA{i}", name=f"hA{i}")
          for i in range(2)]
    hB = [hpool.tile([128, BL], STORE, tag=f"hB{i}", name=f"hB{i}")
          for i in range(2)]

    def gemm_layer(h_in, w_tiles, m_tiles, h_out, bias_cols, e):
        """h_out[mt][:, :] = act(sum_kt w[kt].T @ h_in[kt] + b)."""
        nkt = len(h_in)
        for c in range(NCHUNK):
            for mt in range(m_tiles):
                ps = pspool.tile([128, CH], F32, tag="ps", name="ps")
                for kt in range(nkt):
                    for j in range(NJ):
                        ncol = slice(c * CH + j * NSUB, c * CH + (j + 1) * NSUB)
                        nc.tensor.matmul(
                            ps[:, j * NSUB:(j + 1) * NSUB],
                            _mmv(w_tiles[kt][:, mt * 128:(mt + 1) * 128]),
                            _mmv(h_in[kt][:, ncol]),
                            start=(kt == 0),
                            stop=(kt == nkt - 1),
                            skip_group_check=True,
                        )
                bcol = e * NCONST + bias_cols[mt]
                nc.scalar.activation(
                    _mmv(h_out[mt][:, c * CH:(c + 1) * CH]), ps[:, :],
                    act, bias=cns[:, bcol:bcol + 1],
                )

    # Software pipeline over ensembles: L0(e+1) is emitted between L3(e)
    # and head(e) so the PE always has independent matmul work while the
    # head psums drain on the DVE.
    w_cur = None
    for e in range(E):
        if e == 0:
            w_cur = load_weights(0, first=True)
            gemm_layer([xt], [w_cur[0]], 2, hA, (0, 1), 0)   # L0 of e=0
        w0, w1, w2, w3, wh = w_cur

        # --- the GEMM chain, feature-major ---
        gemm_layer(hA, w1, 2, hB, (2, 3), e)           # 256  -> 256
        gemm_layer(hB, w2, 2, hA, (4, 5), e)           # 256  -> 256
        gemm_layer(hA, w3, 1, hB, (6,), e)             # 256  -> 128 (hB[0])
        h3 = hB[0]

        def head_chunk(c, nsplit):
            cs = slice(c * CH, (c + 1) * CH)
            ps = pspool.tile([64, CH], F32, tag="ps", name="psh")
            for j in range(NJ):
                ncol = slice(c * CH + j * NSUB, c * CH + (j + 1) * NSUB)
                nc.tensor.matmul(
                    ps[:, j * NSUB:(j + 1) * NSUB],
                    _mmv(wh[:, :]), _mmv(h3[:, ncol]),
                    start=True, stop=True,
                )
            # single fused DVE op drains the whole head psum:
            #   rows 0-31:  mu = psum + bmu + state
            #   rows 32-63: y' = psum + (bsig - max) + 0
            bcol = e * NCONST + 7
            hd = sgpool.tile([64, CH], F32, tag="hd", name="hd")
            nc.vector.affine_then_add(
                hd[:, :], ps[:, :], resid[:, cs], 1.0,
                cns[0:64, bcol:bcol + 1],
            )
            step = CH // nsplit
            for p in range(nsplit):
                pcs = slice(c * CH + p * step, c * CH + (p + 1) * step)
                pls = slice(p * step, (p + 1) * step)
                nc.sync.dma_start(io["mu"][e * 32:(e + 1) * 32, pcs],
                                  hd[0:32, pls])
            # pack this member's sigma pre-act into the group tile
            # (32-partition DVE copies may write any quadrant)
            g, r = divmod(e, 4)
            nc.vector.tensor_copy(pk[g][r * 32:(r + 1) * 32, cs], hd[32:64, :])
            if e in (3, E - 1):
                # whole group packed for this chunk: one wide tanh + affine
                # sigmoid(y') = 0.5*tanh(y'/2)+0.5 (Tanh shares Silu's table)
                # sigma = tanh*(exp(max)/2) + (exp(min) + exp(max)/2)
                rows = 128 if g == 0 else 32 * (E - 4)
                sg2 = sgpool.tile([128, CH], F32, tag="sg2", name="sg2", bufs=2)
                nc.scalar.activation(sg2[0:rows, :], pk[g][0:rows, cs],
                                     AF.Tanh, scale=0.5)
                sg3 = sgpool.tile([128, CH], F32, tag="sg3", name="sg3", bufs=2)
                nc.vector.tensor_scalar(
                    sg3[0:rows, :], sg2[0:rows, :],
                    sgc[0:rows, 0:1], sgc[0:rows, 1:2],
                    mybir.AluOpType.mult, mybir.AluOpType.add,
                )
                for p in range(4):
                    pcs = slice(c * CH + p * NSUB, c * CH + (p + 1) * NSUB)
                    pls = slice(p * NSUB, (p + 1) * NSUB)
                    nc.sync.dma_start(io["sig"][g * 128:g * 128 + rows, pcs],
                                      sg3[0:rows, pls])

        # The reference's soft_clamp+exp collapses exactly to
        #   sigma = exp(min) + exp(max)*sigmoid(y - max)
        # head chunk 0 is emitted before L0(e+1), chunk 1 after, so the
        # ACT-side tanh never waits long and the PE never starves.
        nsplit = 2 if e == E - 1 else 1
        head_chunk(0, nsplit)
        if e + 1 < E:
            w_cur = load_weights(e + 1)
            gemm_layer([xt], [w_cur[0]], 2, hA, (0, 1), e + 1)  # L0 of e+1
        head_chunk(1, nsplit)


def build_program(act=AF.Silu):
    nc = bacc.Bacc(
        "TRN2", target_bir_lowering=False, debug=False, num_devices=NCORES
    )
    io = {
        "xt": nc.dram_tensor("xt", [DIN, BL], STORE,
                             kind="ExternalInput").ap(),
        "resid": nc.dram_tensor("resid", [64, BL], F32,
                                kind="ExternalInput").ap(),
        "w0": nc.dram_tensor("w0", [E, DIN, 256], STORE,
                             kind="ExternalInput").ap(),
        "w1": nc.dram_tensor("w1", [E, 256, 256], STORE,
                             kind="ExternalInput").ap(),
        "w2": nc.dram_tensor("w2", [E, 256, 256], STORE,
                             kind="ExternalInput").ap(),
        "w3": nc.dram_tensor("w3", [E, 256, 128], STORE,
                             kind="ExternalInput").ap(),
        "wh": nc.dram_tensor("wh", [E, 128, 64], STORE,
                             kind="ExternalInput").ap(),
        "cns": nc.dram_tensor("cns", [128, E * NCONST], F32,
                              kind="ExternalInput").ap(),
        "sgc": nc.dram_tensor("sgc", [128, 2], F32, kind="ExternalInput").ap(),
        "mu": nc.dram_tensor("mu", [E * 32, BL], F32,
                             kind="ExternalOutput").ap(),
        "sig": nc.dram_tensor("sig", [E * 32, BL], F32,
                              kind="ExternalOutput").ap(),
    }
    with tile.TileContext(nc) as tc, ExitStack() as ctx:
        _build_kernel(ctx, tc, io, act=act)
    nc.compile()
    return nc


def host_prep(state, action, W0, b0, W1, b1, W2, b2, W3, b3,
              Wmu, bmu, Wsig, bsig, max_logstd, min_logstd):
    """Full inputs -> (shared input map, per-core shard maps)."""
    f = lambda a: np.ascontiguousarray(np.asarray(a), dtype=np.float32)
    g = lambda a: np.ascontiguousarray(np.asarray(a, dtype=np.float32)
                                       .astype(NP_STORE))
    state, action = f(state), f(action)
    xt_full = np.ascontiguousarray(
        np.concatenate([state, action], axis=1).T
    )  # [40, B] fp32
    wh = np.concatenate([f(Wmu), f(Wsig)], axis=2)
    b0, b1, b2, b3 = f(b0), f(b1), f(b2), f(b3)
    bmu, bsig = f(bmu), f(bsig)
    mx, mn = f(max_logstd), f(min_logstd)

    cns = np.zeros((128, E * NCONST), np.float32)
    for e in range(E):
        c = e * NCONST
        cns[:, c + 0] = b0[e, :128]
        cns[:, c + 1] = b0[e, 128:]
        cns[:, c + 2] = b1[e, :128]
        cns[:, c + 3] = b1[e, 128:]
        cns[:, c + 4] = b2[e, :128]
        cns[:, c + 5] = b2[e, 128:]
        cns[:, c + 6] = b3[e, :]
        cns[0:32, c + 7] = bmu[e]
        cns[32:64, c + 7] = bsig[e] - mx   # sigma-head drain bias

    sgc = np.zeros((128, 2), np.float32)
    sgc[:, 0] = np.tile(np.exp(mx) / 2, 4)
    sgc[:, 1] = np.tile(np.exp(mn) + np.exp(mx) / 2, 4)

    shared = {
        "w0": g(W0), "w1": g(W1), "w2": g(W2), "w3": g(W3), "wh": g(wh),
        "cns": cns, "sgc": sgc,
    }
    resid_full = np.zeros((64, B), np.float32)
    resid_full[0:32] = xt_full[0:32]
    xt_store = xt_full.astype(NP_STORE)
    shards = [
        {
            "xt": np.ascontiguousarray(xt_store[:, c * BL:(c + 1) * BL]),
            "resid": np.ascontiguousarray(resid_full[:, c * BL:(c + 1) * BL]),
        }
        for c in range(NCORES)
    ]
    return shared, shards


def host_post(results):
    """Per-core {mu,sig} [E*32, BL] -> (mu [E,B,32], sigma [E,B,32])."""
    mu = np.empty((E, B, 32), np.float32)
    sigma = np.empty((E, B, 32), np.float32)
    for c in range(NCORES):
        bs = slice(c * BL, (c + 1) * BL)
        mu[:, bs, :] = results[c]["mu"].reshape(E, 32, BL).transpose(0, 2, 1)
        sigma[:, bs, :] = results[c]["sig"].reshape(E, 32, BL).transpose(0, 2, 1)
    return mu, sigma


_PROGRAM = None


def _get_program():
    global _PROGRAM
    if _PROGRAM is None:
        _PROGRAM = build_program()
    return _PROGRAM


def kernel(**inputs):
    nc = _get_program()
    shared, shards = host_prep(**inputs)
    in_maps = [{**shared, **shards[c]} for c in range(NCORES)]
    res = run_bass_kernel_spmd(nc, in_maps, list(range(NCORES)))
    return host_post(res.results)


# revision 23
# speedup vs baseline: 1.0512x; 1.0038x over previous
"""Trainium2 Bass kernel for nn_EnsembleDynamicModel.

Ensemble MLP: E=7 members, x=[state(32)|action(8)] -> 256 -> 256 -> 256 -> 128
-> {mu(32), log_sigma(32)} with swish hidden activations, soft-clamped
log_sigma -> sigma=exp(.), and mu += state residual.

Strategy: data-parallel over the batch axis. Each of the 8 NeuronCores gets
B/8 = 4096 batch rows; ensemble weights are replicated. On-chip layout keeps
activations feature-major ([feature, batch]) so the contraction dim of every
GEMM sits on SBUF partitions:

    h_out[o, n] = sum_k W[k, o] * h_in[k, n]   (lhsT = W tile, rhs = h_in)

The host pre-transposes x once (cheap, 5 MB) and post-transposes the two
[E*32, B/8] outputs back.

Engines:
  PE   : whole GEMM chain. Storage dtype for weights/activations is bf16 by
         default (full-rate 1 column/cycle at the warm 2.4 GHz clock + fast
         weight load); fp32 storage with float32r matmul views is the
         higher-precision fallback (~427ns per 512-col matmul, SBUF-BW
         limited). PSUM accumulation is fp32 either way.
  ACT  : swish fused with the bias add (Silu(psum + b)); the sigma head's
         sigmoid runs as Tanh (same ACT table set as Silu, so the function
         table never swaps): sigmoid(z) = 0.5*tanh(z/2) + 0.5.
  DVE  : one fused affine_then_add drains each head psum (mu = psum + bmu +
         state on rows 0-31, sigma-preact + (bsig-max) on rows 32-63), plus
         the final sigma scale/offset.

The reference's soft_clamp+exp collapses exactly:
    sigma = exp(min) + exp(max) * sigmoid(y - max).

Ensembles are software-pipelined: L0(e+1) is emitted between L3(e) and
head(e) so the PE always has independent matmul work while head psums drain.
"""

import os
import sys
import numpy as np
from contextlib import ExitStack

# concourse ships with the container image (also on PYTHONPATH via axon_site).
for _p in ("/opt/trn_rl_repo", "/root/.axon_site/_ro/trn_rl_repo"):
    if os.path.isdir(_p) and _p not in sys.path:
        sys.path.append(_p)

import ml_dtypes  # noqa: E402
import concourse.bass as bass  # noqa: E402
import concourse.tile as tile  # noqa: E402
import concourse.mybir as mybir  # noqa: E402
from concourse import bacc  # noqa: E402
from concourse.bass_utils import run_bass_kernel_spmd  # noqa: E402
from concourse import bass_utils as _bu  # noqa: E402

USE_BF16 = True

# Consecutive matmuls here reuse the same stationary weights 4x; walrus's
# LDWEIGHTS dedup (off by default) removes the redundant reloads. Only safe
# for 4-byte weight loads — bf16's fast-weight-load path rejects the opt.
if not USE_BF16 and not getattr(_bu, "_ldw_opt_patched", False):
    _orig_run_command = _bu.run_command

    def _run_command_ldw(argv, **kw):
        argv = ["--enable-ldw-opt=true" if a == "--enable-ldw-opt=false" else a
                for a in argv]
        return _orig_run_command(argv, **kw)

    _bu.run_command = _run_command_ldw
    _bu._ldw_opt_patched = True

F32 = mybir.dt.float32
AF = mybir.ActivationFunctionType

if USE_BF16:
    STORE = mybir.dt.bfloat16      # weights + hidden activations storage
    NP_STORE = ml_dtypes.bfloat16
    _mmv = lambda ap: ap           # matmul reads the tiles natively
else:
    STORE = F32                    # fp32 storage, float32r matmul views
    NP_STORE = np.float32
    _mmv = lambda ap: ap.bitcast(mybir.dt.float32r)

E = 7
B = 32768
S = 32
A = 8
DIN = S + A            # 40
NCORES = 8
BL = B // NCORES       # 4096 batch rows per core
CH = 2048              # batch chunk per psum tile (4 PSUM banks fp32)
NSUB = 512             # one matmul's free dim (1 PSUM bank fp32)
NCHUNK = BL // CH      # 2
NJ = CH // NSUB        # 4
NCONST = 8             # const columns per ensemble member


def _build_kernel(ctx, tc, io, act=AF.Silu):
    nc = tc.nc
    cpool = ctx.enter_context(tc.tile_pool(name="cpool", bufs=1))
    hpool = ctx.enter_context(tc.tile_pool(name="hpool", bufs=1))
    wpool = ctx.enter_context(tc.tile_pool(name="wpool", bufs=2))
    pspool = ctx.enter_context(tc.tile_pool(name="pspool", bufs=2, space="PSUM"))
    sgpool = ctx.enter_context(tc.tile_pool(name="sgpool", bufs=3))

    def load_weights(e, first=False):
        w0 = wpool.tile([DIN, 256], STORE, tag="w0", name="w0")
        nc.sync.dma_start(_mmv(w0[:]), _mmv(io["w0"][e]))
        if first:
            # startup: xt chunk 0 right after w0 so L0 can begin ASAP
            for j in range(NJ):
                js = slice(j * NSUB, (j + 1) * NSUB)
                nc.sync.dma_start(_mmv(xt[:, js]), _mmv(io["xt"][:, js]))
            nc.sync.dma_start(cns[:], io["cns"])
            nc.sync.dma_start(sgc[:], io["sgc"])
        w1, w2, w3 = [], [], []
        for k in range(2):
            t = wpool.tile([128, 256], STORE, tag=f"w1_{k}", name=f"w1_{k}")
            nc.sync.dma_start(_mmv(t[:]),
                              _mmv(io["w1"][e, k * 128:(k + 1) * 128, :]))
            w1.append(t)
            t = wpool.tile([128, 256], STORE, tag=f"w2_{k}", name=f"w2_{k}")
            nc.sync.dma_start(_mmv(t[:]),
                              _mmv(io["w2"][e, k * 128:(k + 1) * 128, :]))
            w2.append(t)
            t = wpool.tile([128, 128], STORE, tag=f"w3_{k}", name=f"w3_{k}")
            nc.sync.dma_start(_mmv(t[:]),
                              _mmv(io["w3"][e, k * 128:(k + 1) * 128, :]))
            w3.append(t)
        wh = wpool.tile([128, 64], STORE, tag="wh", name="wh")
        nc.sync.dma_start(_mmv(wh[:]), _mmv(io["wh"][e]))
        if first:
            for j in range(NJ):
                js = slice(CH + j * NSUB, CH + (j + 1) * NSUB)
                nc.sync.dma_start(_mmv(xt[:, js]), _mmv(io["xt"][:, js]))
            nc.sync.dma_start(resid[:], io["resid"])
        return w0, w1, w2, w3, wh

    scratch = cpool.tile([1, 8], F32, tag="scratch")
    nc.gpsimd.memset(scratch[:], 0.0)
    nc.scalar.activation(scratch[0:1, 0:8], scratch[0:1, 0:8], act, bias=0.0)

    xt = cpool.tile([DIN, BL], STORE, tag="xt")
    cns = cpool.tile([128, E * NCONST], F32, tag="cns")
    sgc = cpool.tile([128, 2], F32, tag="sgc")
    resid = cpool.tile([64, BL], F32, tag="resid")

    # sigma pre-activations packed 4 ensembles per tile: row 32*(e%4)+i
    pk = [sgpool.tile([128, BL], F32, tag=f"pk{g}", name=f"pk{g}", bufs=1)
          for g in range(2)]

    # --- activation buffers, ping-pong between layers ---
    hA = [hpool.tile([128, BL], STORE, tag=f"hA{i}", name=f"hA{i}")
          for i in range(2)]
    hB = [hpool.tile([128, BL], STORE, tag=f"hB{i}", name=f"hB{i}")
          for i in range(2)]

    def gemm_layer(h_in, w_tiles, m_tiles, h_out, bias_cols, e):
        """h_out[mt][:, :] = act(sum_kt w[kt].T @ h_in[kt] + b)."""
        nkt = len(h_in)
        for c in range(NCHUNK):
            for mt in range(m_tiles):
                ps = pspool.tile([128, CH], F32, tag="ps", name="ps")
                for kt in range(nkt):
                    for j in range(NJ):
                        ncol = slice(c * CH + j * NSUB, c * CH + (j + 1) * NSUB)
                        nc.tensor.matmul(
                            ps[:, j * NSUB:(j + 1) * NSUB],
                            _mmv(w_tiles[kt][:, mt * 128:(mt + 1) * 128]),
                            _mmv(h_in[kt][:, ncol]),
                            start=(kt == 0),
                            stop=(kt == nkt - 1),
                            skip_group_check=True,
                        )
                bcol = e * NCONST + bias_cols[mt]
                nc.scalar.activation(
                    _mmv(h_out[mt][:, c * CH:(c + 1) * CH]), ps[:, :],
                    act, bias=cns[:, bcol:bcol + 1],
                )

    # Software pipeline over ensembles: L0(e+1) is emitted between L3(e)
    # and head(e) so the PE always has independent matmul work while the
    # head psums drain on the DVE.
    w_cur = None
    for e in range(E):
        if e == 0:
            w_cur = load_weights(0, first=True)
            gemm_layer([xt], [w_cur[0]], 2, hA, (0, 1), 0)   # L0 of e=0
        w0, w1, w2, w3, wh = w_cur

        # --- the GEMM chain, feature-major ---
        gemm_layer(hA, w1, 2, hB, (2, 3), e)           # 256  -> 256
        gemm_layer(hB, w2, 2, hA, (4, 5), e)           # 256  -> 256
        gemm_layer(hA, w3, 1, hB, (6,), e)             # 256  -> 128 (hB[0])
        h3 = hB[0]

        def head_chunk(c, nsplit):
            cs = slice(c * CH, (c + 1) * CH)
            ps = pspool.tile([64, CH], F32, tag="ps", name="psh")
            for j in range(NJ):
                ncol = slice(c * CH + j * NSUB, c * CH + (j + 1) * NSUB)
                nc.tensor.matmul(
                    ps[:, j * NSUB:(j + 1) * NSUB],
                    _mmv(wh[:, :]), _mmv(h3[:, ncol]),
                    start=True, stop=True,
                )
            # single fused DVE op drains the whole head psum:
            #   rows 0-31:  mu = psum + bmu + state
            #   rows 32-63: y' = psum + (bsig - max) + 0
            bcol = e * NCONST + 7
            hd = sgpool.tile([64, CH], F32, tag="hd", name="hd")
            nc.vector.affine_then_add(
                hd[:, :], ps[:, :], resid[:, cs], 1.0,
                cns[0:64, bcol:bcol + 1],
            )
            step = CH // nsplit
            for p in range(nsplit):
                pcs = slice(c * CH + p * step, c * CH + (p + 1) * step)
                pls = slice(p * step, (p + 1) * step)
                nc.sync.dma_start(io["mu"][e * 32:(e + 1) * 32, pcs],
                                  hd[0:32, pls])
            # sigmoid via tanh (Silu's table set): s = 0.5*tanh(y'/2)+0.5,
            # sigma = tanh*(exp(max)/2) + (exp(min) + exp(max)/2).
            # Members are packed 4-wide so the tanh uses all 128 ACT lanes;
            # groups flush incrementally (e3: rows 0-128 of group 0; e5:
            # rows 0-64 of group 1) and the final member takes a direct
            # unpacked path so the kernel tail skips the pack-copy.
            g, r = divmod(e, 4)
            if e == E - 1:
                sg2 = sgpool.tile([64, CH], F32, tag="sg2e", name="sg2e",
                                  bufs=2)
                nc.scalar.activation(sg2[32:64, :], hd[32:64, :], AF.Tanh,
                                     scale=0.5)
                sg3 = sgpool.tile([64, CH], F32, tag="sg3e", name="sg3e",
                                  bufs=2)
                nc.vector.tensor_scalar(
                    sg3[32:64, :], sg2[32:64, :],
                    sgc[32:64, 0:1], sgc[32:64, 1:2],
                    mybir.AluOpType.mult, mybir.AluOpType.add,
                )
                for p in range(4):
                    pcs = slice(c * CH + p * NSUB, c * CH + (p + 1) * NSUB)
                    pls = slice(p * NSUB, (p + 1) * NSUB)
                    nc.sync.dma_start(io["sig"][e * 32:(e + 1) * 32, pcs],
                                      sg3[32:64, pls])
            else:
                # 32-partition DVE copies may write any quadrant
                nc.vector.tensor_copy(pk[g][r * 32:(r + 1) * 32, cs],
                                      hd[32:64, :])
            if e in (3, 5):
                rows = 128 if e == 3 else 64
                sg2 = sgpool.tile([128, CH], F32, tag="sg2", name="sg2", bufs=2)
                nc.scalar.activation(sg2[0:rows, :], pk[g][0:rows, cs],
                                     AF.Tanh, scale=0.5)
                sg3 = sgpool.tile([128, CH], F32, tag="sg3", name="sg3", bufs=2)
                nc.vector.tensor_scalar(
                    sg3[0:rows, :], sg2[0:rows, :],
                    sgc[0:rows, 0:1], sgc[0:rows, 1:2],
                    mybir.AluOpType.mult, mybir.AluOpType.add,
                )
                for p in range(4):
                    pcs = slice(c * CH + p * NSUB, c * CH + (p + 1) * NSUB)
                    pls = slice(p * NSUB, (p + 1) * NSUB)
                    nc.sync.dma_start(io["sig"][g * 128:g * 128 + rows, pcs],
                                      sg3[0:rows, pls])

        # The reference's soft_clamp+exp collapses exactly to
        #   sigma = exp(min) + exp(max)*sigmoid(y - max)
        # head chunk 0 is emitted before L0(e+1), chunk 1 after, so the
        # ACT-side tanh never waits long and the PE never starves.
        nsplit = 2 if e == E - 1 else 1
        head_chunk(0, nsplit)
        if e + 1 < E:
            w_cur = load_weights(e + 1)
            gemm_layer([xt], [w_cur[0]], 2, hA, (0, 1), e + 1)  # L0 of e+1
        head_chunk(1, nsplit)


def build_program(act=AF.Silu):
    nc = bacc.Bacc(
        "TRN2", target_bir_lowering=False, debug=False, num_devices=NCORES
    )
    io = {
        "xt": nc.dram_tensor("xt", [DIN, BL], STORE,
                             kind="ExternalInput").ap(),
        "resid": nc.dram_tensor("resid", [64, BL], F32,
                                kind="ExternalInput").ap(),
        "w0": nc.dram_tensor("w0", [E, DIN, 256], STORE,
                             kind="ExternalInput").ap(),
        "w1": nc.dram_tensor("w1", [E, 256, 256], STORE,
                             kind="ExternalInput").ap(),
        "w2": nc.dram_tensor("w2", [E, 256, 256], STORE,
                             kind="ExternalInput").ap(),
        "w3": nc.dram_tensor("w3", [E, 256, 128], STORE,
                             kind="ExternalInput").ap(),
        "wh": nc.dram_tensor("wh", [E, 128, 64], STORE,
                             kind="ExternalInput").ap(),
        "cns": nc.dram_tensor("cns", [128, E * NCONST], F32,
                              kind="ExternalInput").ap(),
        "sgc": nc.dram_tensor("sgc", [128, 2], F32, kind="ExternalInput").ap(),
        "mu": nc.dram_tensor("mu", [E * 32, BL], F32,
                             kind="ExternalOutput").ap(),
        "sig": nc.dram_tensor("sig", [E * 32, BL], F32,
                              kind="ExternalOutput").ap(),
    }
    with tile.TileContext(nc) as tc, ExitStack() as ctx:
        _build_kernel(ctx, tc, io, act=act)
    nc.compile()
    return nc


def host_prep(state, action, W0, b0, W1, b1, W2, b2, W3, b3,
              Wmu, bmu, Wsig, bsig, max_logstd, min_logstd):
    """Full inputs -> (shared input map, per-core shard maps)."""
    f = lambda a: np.ascontiguousarray(np.asarray(a), dtype=np.float32)
    g = lambda a: np.ascontiguousarray(np.asarray(a, dtype=np.float32)
                                       .astype(NP_STORE))
    state, action = f(state), f(action)
    xt_full = np.ascontiguousarray(
        np.concatenate([state, action], axis=1).T
    )  # [40, B] fp32
    wh = np.concatenate([f(Wmu), f(Wsig)], axis=2)
    b0, b1, b2, b3 = f(b0), f(b1), f(b2), f(b3)
    bmu, bsig = f(bmu), f(bsig)
    mx, mn = f(max_logstd), f(min_logstd)

    cns = np.zeros((128, E * NCONST), np.float32)
    for e in range(E):
        c = e * NCONST
        cns[:, c + 0] = b0[e, :128]
        cns[:, c + 1] = b0[e, 128:]
        cns[:, c + 2] = b1[e, :128]
        cns[:, c + 3] = b1[e, 128:]
        cns[:, c + 4] = b2[e, :128]
        cns[:, c + 5] = b2[e, 128:]
        cns[:, c + 6] = b3[e, :]
        cns[0:32, c + 7] = bmu[e]
        cns[32:64, c + 7] = bsig[e] - mx   # sigma-head drain bias

    sgc = np.zeros((128, 2), np.float32)
    sgc[:, 0] = np.tile(np.exp(mx) / 2, 4)
    sgc[:, 1] = np.tile(np.exp(mn) + np.exp(mx) / 2, 4)

    shared = {
        "w0": g(W0), "w1": g(W1), "w2": g(W2), "w3": g(W3), "wh": g(wh),
        "cns": cns, "sgc": sgc,
    }
    resid_full = np.zeros((64, B), np.float32)
    resid_full[0:32] = xt_full[0:32]
    xt_store = xt_full.astype(NP_STORE)
    shards = [
        {
            "xt": np.ascontiguousarray(xt_store[:, c * BL:(c + 1) * BL]),
            "resid": np.ascontiguousarray(resid_full[:, c * BL:(c + 1) * BL]),
        }
        for c in range(NCORES)
    ]
    return shared, shards


def host_post(results):
    """Per-core {mu,sig} [E*32, BL] -> (mu [E,B,32], sigma [E,B,32])."""
    mu = np.empty((E, B, 32), np.float32)
    sigma = np.empty((E, B, 32), np.float32)
    for c in range(NCORES):
        bs = slice(c * BL, (c + 1) * BL)
        mu[:, bs, :] = results[c]["mu"].reshape(E, 32, BL).transpose(0, 2, 1)
        sigma[:, bs, :] = results[c]["sig"].reshape(E, 32, BL).transpose(0, 2, 1)
    return mu, sigma


_PROGRAM = None


def _get_program():
    global _PROGRAM
    if _PROGRAM is None:
        _PROGRAM = build_program()
    return _PROGRAM


def kernel(**inputs):
    nc = _get_program()
    shared, shards = host_prep(**inputs)
    in_maps = [{**shared, **shards[c]} for c in range(NCORES)]
    res = run_bass_kernel_spmd(nc, in_maps, list(range(NCORES)))
    return host_post(res.results)
